# revision 21
# baseline (speedup 1.0000x reference)
"""Trainium2 Bass kernel for nn_CrossAttnBlock (B=4, Lq=Lk=2048, D=1024, H=16, Dh=64).

Sharding: 8 cores = (batch b in 0..3) x (query half in 0..1). Each core computes
cross-attention for 1024 query tokens of one batch against that batch's full
2048-token context. K/V work is duplicated across the two cores of a batch
(zero collectives needed).

All matmul operands are bf16 (PSUM accumulates fp32); this hardware duty-cycle
throttles the PE to ~50% under sustained load, so the program minimizes PE
cycles and keeps the PE stream gap-free:
  - token->feature transposes on the DMA crossbar (dma_start_transpose): the
    PE does zero transposes and the DVE does zero PSUM evictions (ACT evicts)
  - LN -> projection loops are software-pipelined in stages (A: load+LN+xbar,
    B: matmuls+evict, C: l2norm+xbar) with A emitted two tiles ahead of B, so
    the in-order DVE/ACT queues never serialize against the PE (each
    cross-engine hop costs ~1us of latency)
  - attention S/O software-pipelined: O(t-1) emitted after S(t) so the PE
    never waits on the ACT exp; per-head softmax normalization (DVE reciprocal
    + ones-matmul partition broadcast + multiply) is deferred into the middle
    of the next head's loop so its PE matmul never stalls the stream
  - weights and the first ctx tiles prefetched across phase boundaries

rsqrt is computed as exp(-0.5*ln(s)) (+ one Newton step for the l2 norms; the
LN rsqrt skips Newton since a per-token scale error cancels in the normalized
q-hat/k-hat). Softmax needs no max-subtraction: |scores/tau'| <= 8.01.
"""

import contextlib

import numpy as np

import bass_rust
import concourse.bass as bass
import concourse.tile as tile
from concourse import mybir
from concourse.bass_utils import run_bass_kernel_spmd

F32 = mybir.dt.float32
F32R = mybir.dt.float32r
BF16 = mybir.dt.bfloat16
AF = mybir.ActivationFunctionType
ALU = mybir.AluOpType

B, LQ, LK = 4, 2048, 2048
DQ, DC = 1024, 1024
H, DH = 16, 64
INNER = H * DH
LN_EPS = 1e-5

LQS = LQ // 2          # query tokens per core (1024)
NQT = LQS // 128       # 8 query token tiles
NKT = LK // 128        # 16 k token tiles
NF = DQ // 128         # 8 feature tiles


def _rsqrt(nc, pool, s_ap, out_ap, newton=True):
    """out = 1/sqrt(s) via exp(-0.5*ln(s)), optionally one Newton step."""
    p, n = s_ap.shape[0], s_ap.free_size()
    t = pool.tile([p, n], F32, tag="rsq_t")
    nc.scalar.activation(out=t[:, :n], in_=s_ap, func=AF.Ln)
    nc.scalar.activation(out=out_ap, in_=t[:, :n], func=AF.Exp, scale=-0.5)
    if newton:
        # r = r * (1.5 - 0.5 * s * r^2)
        a = pool.tile([p, n], F32, tag="rsq_a")
        nc.vector.tensor_mul(out=a[:, :n], in0=out_ap, in1=out_ap)
        nc.vector.tensor_mul(out=a[:, :n], in0=a[:, :n], in1=s_ap)
        nc.vector.tensor_scalar(
            out=a[:, :n], in0=a[:, :n], scalar1=-0.5, scalar2=1.5,
            op0=ALU.mult, op1=ALU.add,
        )
        nc.vector.tensor_mul(out=out_ap, in0=out_ap, in1=a[:, :n])


def _layernorm_tile(nc, pool, x_ap, z_ap):
    """z = (x - mean) * rsqrt(var + eps) for a [128, 1024] token-major tile.

    No Newton step on the rsqrt: the resulting per-token scale error is
    common-mode and cancels exactly in the l2-normalized q-hat/k-hat; only v
    inherits it (~1e-3), far inside the error budget. Keeps the serial
    cross-engine chain short (each hop costs ~1us of latency)."""
    p = x_ap.shape[0]
    stats = pool.tile([p, 2, 6], F32, tag="ln_stats")
    x3 = x_ap.rearrange("p (c f) -> p c f", c=2)
    for c in range(2):
        nc.vector.bn_stats(out=stats[:, c, :], in_=x3[:, c, :])
    mv = pool.tile([p, 2], F32, tag="ln_mv")
    nc.vector.bn_aggr(out=mv[:], in_=stats[:])
    s = pool.tile([p, 1], F32, tag="ln_s")
    nc.vector.tensor_scalar(
        out=s[:], in0=mv[:, 1:2], scalar1=LN_EPS, scalar2=None, op0=ALU.add,
    )
    inv = pool.tile([p, 1], F32, tag="ln_inv")
    _rsqrt(nc, pool, s[:], inv[:], newton=False)
    nc.vector.tensor_scalar(
        out=z_ap, in0=x_ap, scalar1=mv[:, 0:1], scalar2=inv[:],
        op0=ALU.subtract, op1=ALU.mult,
    )


def build_program(inv_tau: float, has_bias: bool):
    """Build the single-core SPMD bass program."""
    nc = bass.Bass()

    xs = nc.declare_dram_parameter("xs", [LQS, DQ], BF16, isOutput=False)
    ctx = nc.declare_dram_parameter("ctx", [LK, DC], BF16, isOutput=False)
    mask01 = nc.declare_dram_parameter("mask01", [LK], F32, isOutput=False)
    wq = nc.declare_dram_parameter("wq", [DQ, INNER], BF16, isOutput=False)
    wk = nc.declare_dram_parameter("wk", [DC, INNER], BF16, isOutput=False)
    wv = nc.declare_dram_parameter("wv", [DC, INNER], BF16, isOutput=False)
    wo = nc.declare_dram_parameter("wo", [INNER, DQ], BF16, isOutput=False)
    if has_bias:
        cq = nc.declare_dram_parameter("cq", [INNER], F32, isOutput=False)
        ck = nc.declare_dram_parameter("ck", [INNER], F32, isOutput=False)
        cv = nc.declare_dram_parameter("cv", [INNER], F32, isOutput=False)
    out = nc.declare_dram_parameter("out", [LQS, DQ], F32, isOutput=True)

    with tile.TileContext(nc) as tc:
        with contextlib.ExitStack() as stack:
            consts = stack.enter_context(tc.tile_pool(name="consts", bufs=1))
            ones_f = consts.tile([1, DH], F32)
            nc.vector.memset(ones_f[:], 1.0)
            ones_r = consts.tile([1, DH], F32R)
            nc.vector.tensor_copy(out=ones_r[:], in_=ones_f[:])
            ones_dh = ones_r[:]

            # mask as [128, NKT] float (token t*128+p at [p, t])
            mask_sb = consts.tile([128, NKT], F32)
            nc.sync.dma_start(
                out=mask_sb[:], in_=mask01.rearrange("(t p) -> p t", p=128)
            )
            if has_bias:
                cq_b = consts.tile([128, INNER], F32)
                ck_b = consts.tile([128, INNER], F32)
                cv_b = consts.tile([128, INNER], F32)
                for dst, src in ((cq_b, cq), (ck_b, ck), (cv_b, cv)):
                    bc = bass.AP(
                        tensor=src.tensor, offset=src.offset,
                        ap=[[0, 128]] + list(src.ap),
                    )
                    nc.gpsimd.dma_start(out=dst[:], in_=bc)

            small = stack.enter_context(tc.tile_pool(name="small", bufs=2))

            # persistent feature-major activations (bf16)
            persist = stack.enter_context(tc.tile_pool(name="persist", bufs=1))
            qhatT = persist.tile([128, NF, LQS], BF16, name="qhatT")
            oaT = persist.tile([128, NF, LQS], BF16, name="oaT")

            # kT/vpp persist into P2b (entered before kv era: LIFO order)
            kvw_pool = stack.enter_context(tc.tile_pool(name="kvw", bufs=1))
            kT = kvw_pool.tile([128, NF, LK], BF16, name="kT")
            vpp = kvw_pool.tile([128, NKT, H, DH + 1], BF16, name="vpp")
            # preset all mask columns once: vpp[:, t, h, 64] = mask[:, t]
            nc.gpsimd.tensor_copy(
                out=vpp[:, :, :, DH:DH + 1],
                in_=mask_sb[:].unsqueeze(2).broadcast_to([128, NKT, H]
                                                        ).unsqueeze(3),
            )

            with contextlib.ExitStack() as kv_era:
                kvin_pool = kv_era.enter_context(
                    tc.tile_pool(name="kvin", bufs=1)
                )
                kvT = kvin_pool.tile([128, NF, LK], BF16, name="kvT")
                ln_pool = kv_era.enter_context(
                    tc.tile_pool(name="ln", bufs=4)
                )

                def ctx_stage_a(t):
                    xt = ln_pool.tile([128, DQ], BF16, tag="ln_x",
                                      name=f"kx{t}")
                    nc.sync.dma_start(
                        out=xt[:], in_=ctx[t * 128:(t + 1) * 128, :]
                    )
                    zt = ln_pool.tile([128, DQ], BF16, tag="ln_z",
                                      name=f"kz{t}")
                    _layernorm_tile(nc, small, xt[:], zt[:])
                    nc.sync.dma_start_transpose(
                        out=kvT[:, :, t * 128:(t + 1) * 128], in_=zt[:]
                    )

                # ---- P1: x LayerNorm + q projection ----
                # Software-pipelined stages: A(t)=load+LN+xbar, B(t)=matmuls+
                # evict, C(t)=l2norm+xbar. Emitting A(t) before B(t-1) before
                # C(t-2) keeps the in-order engine queues from serializing the
                # per-tile cross-engine chains against the PE.
                with contextlib.ExitStack() as p1:
                    wq_pool = p1.enter_context(tc.tile_pool(name="wq", bufs=1))
                    wq_sb = wq_pool.tile([128, NF, INNER], BF16)
                    qin_pool = p1.enter_context(tc.tile_pool(name="qin", bufs=1))
                    qinT = qin_pool.tile([128, NF, LQS], BF16, name="qinT")
                    mm_psum = p1.enter_context(
                        tc.tile_pool(name="mm_psum1", bufs=2, space="PSUM")
                    )
                    qt_pool = p1.enter_context(tc.tile_pool(name="qtok", bufs=3))
                    qsq_pool = p1.enter_context(
                        tc.tile_pool(name="qsq", bufs=2)
                    )
                    qtok_of = {}

                    def q_stage_a(t):
                        xt = ln_pool.tile([128, DQ], BF16, tag="ln_x",
                                          name=f"qx{t}")
                        nc.sync.dma_start(
                            out=xt[:], in_=xs[t * 128:(t + 1) * 128, :]
                        )
                        zt = ln_pool.tile([128, DQ], BF16, tag="ln_z",
                                          name=f"qz{t}")
                        _layernorm_tile(nc, small, xt[:], zt[:])
                        nc.sync.dma_start_transpose(
                            out=qinT[:, :, t * 128:(t + 1) * 128], in_=zt[:]
                        )

                    def q_stage_b(t):
                        qtok = qt_pool.tile([128, INNER], BF16, tag="qtok",
                                            name=f"qtok{t}")
                        qtok_of[t] = qtok
                        ps = mm_psum.tile([128, INNER], F32, tag="mm",
                                          name=f"qps{t}")
                        for n in range(2):
                            for fi in range(NF):
                                nc.tensor.matmul(
                                    ps[:, n * 512:(n + 1) * 512],
                                    qinT[:, fi, t * 128:(t + 1) * 128],
                                    wq_sb[:, fi, n * 512:(n + 1) * 512],
                                    start=(fi == 0),
                                    stop=(fi == NF - 1),
                                )
                        if has_bias:
                            nc.vector.tensor_add(
                                out=qtok[:], in0=ps[:], in1=cq_b[:]
                            )
                        else:
                            nc.scalar.copy(out=qtok[:], in_=ps[:])

                    def q_stage_c(t):
                        qtok = qtok_of.pop(t)
                        sq = qsq_pool.tile([128, INNER], BF16, tag="qsq",
                                           name=f"qsq{t}")
                        nc.vector.tensor_mul(
                            out=sq[:], in0=qtok[:], in1=qtok[:]
                        )
                        ssq = small.tile([128, H], F32, tag="qssq")
                        nc.vector.tensor_reduce(
                            out=ssq[:],
                            in_=sq[:].rearrange("p (h d) -> p h d", h=H),
                            axis=mybir.AxisListType.X,
                            op=ALU.add,
                        )
                        rq = small.tile([128, H], F32, tag="qrq")
                        _rsqrt(nc, small, ssq[:], rq[:])
                        q3 = qtok[:].rearrange("p (h d) -> p h d", h=H)
                        nc.vector.tensor_tensor(
                            out=q3,
                            in0=q3,
                            in1=rq[:].unsqueeze(2).broadcast_to([128, H, DH]),
                            op=ALU.mult,
                        )
                        nc.sync.dma_start_transpose(
                            out=qhatT[:, :, t * 128:(t + 1) * 128], in_=qtok[:]
                        )

                    # x0's LN starts before the wq weight load so the DMA
                    # queue delivers the first tile immediately.
                    q_stage_a(0)
                    for fi in range(NF):
                        nc.sync.dma_start(
                            out=wq_sb[:, fi, :],
                            in_=wq[fi * 128:(fi + 1) * 128, :],
                        )
                    for s in range(1, NQT + 4):
                        if s < NQT:
                            q_stage_a(s)
                        if s == NQT:
                            ctx_stage_a(0)
                        if s == NQT + 1:
                            ctx_stage_a(1)
                        if 3 <= s < NQT + 3:
                            q_stage_b(s - 3)
                        if s >= 4:
                            q_stage_c(s - 4)

                # ---- P2a: ctx LayerNorm + k/v projection, merged per tile ----
                # kT[:, fi, :] holds features [fi*128, (fi+1)*128) = heads
                # (2*fi, 2*fi+1) stacked 64+64 on partitions.
                with contextlib.ExitStack() as p2a:
                    wkv_pool = p2a.enter_context(
                        tc.tile_pool(name="wkv", bufs=1)
                    )
                    wk_sb = wkv_pool.tile([128, NF, INNER], BF16, name="wk_sb")
                    wv_sb = wkv_pool.tile([128, NF, INNER], BF16, name="wv_sb")
                    for fi in range(NF):
                        nc.sync.dma_start(
                            out=wk_sb[:, fi, :],
                            in_=wk[fi * 128:(fi + 1) * 128, :],
                        )
                        nc.sync.dma_start(
                            out=wv_sb[:, fi, :],
                            in_=wv[fi * 128:(fi + 1) * 128, :],
                        )
                    ln_pool = p2a.enter_context(tc.tile_pool(name="ln2", bufs=3))
                    mm_psum = p2a.enter_context(
                        tc.tile_pool(name="mm_psum2", bufs=2, space="PSUM")
                    )
                    kt_pool = p2a.enter_context(
                        tc.tile_pool(name="ktok", bufs=3)
                    )
                    ksq_pool = p2a.enter_context(
                        tc.tile_pool(name="ksq", bufs=2)
                    )
                    ktok_of = {}
                    ps_v_of = {}

                    def k_stage_b(t):
                        ps_k = mm_psum.tile([128, INNER], F32, tag="mm_k",
                                            name=f"psk{t}")
                        ps_v = mm_psum.tile([128, INNER], F32, tag="mm_v",
                                            name=f"psv{t}")
                        ps_v_of[t] = ps_v
                        for n in range(2):
                            for fi in range(NF):
                                nc.tensor.matmul(
                                    ps_k[:, n * 512:(n + 1) * 512],
                                    kvT[:, fi, t * 128:(t + 1) * 128],
                                    wk_sb[:, fi, n * 512:(n + 1) * 512],
                                    start=(fi == 0), stop=(fi == NF - 1),
                                )
                        ktok = kt_pool.tile([128, INNER], BF16, tag="ktok",
                                            name=f"ktok{t}")
                        ktok_of[t] = ktok
                        if has_bias:
                            nc.vector.tensor_add(
                                out=ktok[:], in0=ps_k[:], in1=ck_b[:]
                            )
                        else:
                            nc.scalar.copy(out=ktok[:], in_=ps_k[:])
                        for n in range(2):
                            for fi in range(NF):
                                nc.tensor.matmul(
                                    ps_v[:, n * 512:(n + 1) * 512],
                                    kvT[:, fi, t * 128:(t + 1) * 128],
                                    wv_sb[:, fi, n * 512:(n + 1) * 512],
                                    start=(fi == 0), stop=(fi == NF - 1),
                                )

                    def k_stage_c(t):
                        ktok = ktok_of.pop(t)
                        ps_v = ps_v_of.pop(t)
                        # V'' = [v * mask | mask] (mask col preset above)
                        mt = mask_sb[:, t:t + 1]
                        if has_bias:
                            vtmp = kt_pool.tile([128, INNER], F32, tag="vtmp",
                                                name=f"vtmp{t}")
                            nc.vector.tensor_add(
                                out=vtmp[:], in0=ps_v[:], in1=cv_b[:]
                            )
                            vsrc = vtmp[:]
                        else:
                            vsrc = ps_v[:]
                        nc.vector.tensor_scalar_mul(
                            out=vpp[:, t, :, 0:DH],
                            in0=vsrc.rearrange("p (h d) -> p h d", h=H),
                            scalar1=mt,
                        )
                        # k l2 norm (in place) then xbar transpose
                        sqk = ksq_pool.tile([128, INNER], BF16, tag="ksq",
                                            name=f"ksq{t}")
                        nc.vector.tensor_mul(
                            out=sqk[:], in0=ktok[:], in1=ktok[:]
                        )
                        ssqk = small.tile([128, H], F32, tag="kssq")
                        nc.vector.tensor_reduce(
                            out=ssqk[:],
                            in_=sqk[:].rearrange("p (h d) -> p h d", h=H),
                            axis=mybir.AxisListType.X,
                            op=ALU.add,
                        )
                        rk = small.tile([128, H], F32, tag="krk")
                        _rsqrt(nc, small, ssqk[:], rk[:])
                        k3 = ktok[:].rearrange("p (h d) -> p h d", h=H)
                        nc.vector.tensor_tensor(
                            out=k3,
                            in0=k3,
                            in1=rk[:].unsqueeze(2).broadcast_to([128, H, DH]),
                            op=ALU.mult,
                        )
                        nc.sync.dma_start_transpose(
                            out=kT[:, :, t * 128:(t + 1) * 128], in_=ktok[:]
                        )

                    for s in range(2, NKT + 3):
                        if s < NKT:
                            ctx_stage_a(s)
                        if s < NKT + 2:
                            k_stage_b(s - 2)
                        if s >= 3:
                            k_stage_c(s - 3)

            # ---- P2b: attention per head, S/O software-pipelined ----
            wo_pool = stack.enter_context(tc.tile_pool(name="wo", bufs=1))
            wo_sb = wo_pool.tile([128, NF, DQ], BF16)
            for fi in range(NF):
                nc.sync.dma_start(
                    out=wo_sb[:, fi, :], in_=wo[fi * 128:(fi + 1) * 128, :]
                )
            with contextlib.ExitStack() as p2b:
                exp_pool = p2b.enter_context(tc.tile_pool(name="exp", bufs=4))
                rec_pool = p2b.enter_context(tc.tile_pool(name="rec", bufs=2))
                s_psum = p2b.enter_context(
                    tc.tile_pool(name="s_psum", bufs=2, space="PSUM")
                )
                ps_o_pool = p2b.enter_context(
                    tc.tile_pool(name="ps_o", bufs=2, space="PSUM")
                )
                ps_o_of = {}
                es_of = {}

                def emit_normalize(g):
                    """Denominator reciprocal + partition-broadcast + scale."""
                    gfi, gr = g // 2, (g % 2) * DH
                    ps_o = ps_o_of.pop(g)
                    rec_f = rec_pool.tile([1, LQS], F32, tag="rec_f")
                    nc.vector.reciprocal(out=rec_f[:], in_=ps_o[DH:DH + 1, :])
                    recr = rec_pool.tile([1, LQS], F32R, tag="recr")
                    nc.vector.tensor_copy(out=recr[:], in_=rec_f[:])
                    rb_ps = s_psum.tile([DH, LQS], F32, tag="mm_s")
                    for c in range(2):
                        nc.tensor.matmul(
                            rb_ps[:, c * 512:(c + 1) * 512],
                            ones_dh,
                            recr[0:1, c * 512:(c + 1) * 512],
                            start=True,
                            stop=True,
                        )
                    recb = rec_pool.tile([DH, LQS], F32, tag="recb")
                    nc.vector.tensor_copy(out=recb[:], in_=rb_ps[:])
                    nc.vector.tensor_tensor(
                        out=oaT[gr:gr + DH, gfi, :],
                        in0=ps_o[0:DH, :],
                        in1=recb[:],
                        op=ALU.mult,
                    )

                def emit_o(g, t):
                    gfi, gr = g // 2, (g % 2) * DH
                    for c in range(2):
                        nc.tensor.matmul(
                            ps_o_of[g][:, c * 512:(c + 1) * 512],
                            vpp[:, t, g, :],
                            es_of[(g, t)][:, c * 512:(c + 1) * 512],
                            start=(t == 0),
                            stop=(t == NKT - 1),
                        )
                    del es_of[(g, t)]

                for g in range(H):
                    gfi, gr = g // 2, (g % 2) * DH
                    ps_o_of[g] = ps_o_pool.tile(
                        [DH + 1, LQS], F32, tag="ps_o", name=f"ps_o{g}"
                    )
                    for t in range(NKT):
                        ps_s = s_psum.tile([128, LQS], F32, tag="mm_s")
                        for c in range(2):
                            nc.tensor.matmul(
                                ps_s[:, c * 512:(c + 1) * 512],
                                kT[gr:gr + DH, gfi, t * 128:(t + 1) * 128],
                                qhatT[gr:gr + DH, gfi, c * 512:(c + 1) * 512],
                                start=True,
                                stop=True,
                            )
                        es = exp_pool.tile([128, LQS], BF16, tag="es")
                        nc.scalar.activation(
                            out=es[:], in_=ps_s[:], func=AF.Exp, scale=inv_tau,
                        )
                        es_of[(g, t)] = es
                        if t >= 1:
                            emit_o(g, t - 1)
                        if t == 6 and g >= 1:
                            emit_normalize(g - 1)
                    emit_o(g, NKT - 1)
                emit_normalize(H - 1)

            # ---- P3: output projection ----
            with contextlib.ExitStack() as p3:
                mm_psum = p3.enter_context(
                    tc.tile_pool(name="mm_psum3", bufs=3, space="PSUM")
                )
                fin_pool = p3.enter_context(tc.tile_pool(name="fin", bufs=3))
                for t in range(NQT):
                    ft = fin_pool.tile([128, DQ], F32, tag="fin")
                    for n in range(2):
                        ps = mm_psum.tile([128, 512], F32, tag="mm")
                        for fi in range(NF):
                            nc.tensor.matmul(
                                ps[:],
                                oaT[:, fi, t * 128:(t + 1) * 128],
                                wo_sb[:, fi, n * 512:(n + 1) * 512],
                                start=(fi == 0),
                                stop=(fi == NF - 1),
                            )
                        nc.scalar.copy(
                            out=ft[:, n * 512:(n + 1) * 512], in_=ps[:]
                        )
                    nc.sync.dma_start(
                        out=out[t * 128:(t + 1) * 128, :], in_=ft[:]
                    )

    return nc


def split_multi_waits(nc):
    """walrus in this environment rejects >1 sync wait per instruction; move
    extras onto same-engine NOPs immediately preceding the instruction."""
    ctr = 0
    for f in nc.m.functions:
        for bb in f.blocks:
            new = []
            for inst in bb.instructions:
                si = inst.sync_info
                if si is not None and len(si.on_wait) > 1:
                    waits = list(si.on_wait)
                    for w in waits[:-1]:
                        nop = bass_rust.InstNoOp(name=f"I-wsplit-{ctr}")
                        ctr += 1
                        nop.engine = inst.engine
                        nop.sync_info = bass_rust.SyncInfo(
                            on_wait=[w], on_update=[]
                        )
                        new.append(nop)
                    inst.sync_info = bass_rust.SyncInfo(
                        on_wait=[waits[-1]], on_update=list(si.on_update)
                    )
                new.append(inst)
            bb.instructions[:] = new
    return ctr


_PROGRAM_CACHE = {}


def _get_program(inv_tau: float, has_bias: bool):
    key = (round(float(inv_tau), 12), has_bias)
    if key not in _PROGRAM_CACHE:
        nc = build_program(float(inv_tau), has_bias)
        split_multi_waits(nc)
        _PROGRAM_CACHE[key] = nc
    return _PROGRAM_CACHE[key]


def make_core_inputs(x, context, key_padding_mask, ln_q_w, ln_q_b, ln_ctx_w,
                     ln_ctx_b, Wq, Wk, Wv, Wo, tau):
    """Shard + host-side weight folding. Returns (in_maps, has_bias)."""
    import ml_dtypes

    f32 = np.float32
    bf16 = ml_dtypes.bfloat16
    x = np.asarray(x, f32)
    context = np.asarray(context, f32)
    mask01 = 1.0 - np.asarray(key_padding_mask).astype(f32)  # 1 = keep
    ln_q_w = np.asarray(ln_q_w, f32)
    ln_q_b = np.asarray(ln_q_b, f32)
    ln_ctx_w = np.asarray(ln_ctx_w, f32)
    ln_ctx_b = np.asarray(ln_ctx_b, f32)
    Wq = np.asarray(Wq, f32)
    Wk = np.asarray(Wk, f32)
    Wv = np.asarray(Wv, f32)
    Wo = np.asarray(Wo, f32)

    # fold LN affine into projections: (z*w + b) @ W = z @ (w*W) + b@W
    wq_f = np.ascontiguousarray(Wq * ln_q_w[:, None]).astype(bf16)
    wk_f = np.ascontiguousarray(Wk * ln_ctx_w[:, None]).astype(bf16)
    wv_f = np.ascontiguousarray(Wv * ln_ctx_w[:, None]).astype(bf16)
    wo_f = Wo.astype(bf16)
    has_bias = bool(np.any(ln_q_b != 0.0) or np.any(ln_ctx_b != 0.0))
    cq = (ln_q_b @ Wq).astype(f32)
    ck = (ln_ctx_b @ Wk).astype(f32)
    cv = (ln_ctx_b @ Wv).astype(f32)

    x_b = x.astype(bf16)
    ctx_b = context.astype(bf16)

    in_maps = []
    for core in range(8):
        b, hq = core // 2, core % 2
        m = {
            "xs": np.ascontiguousarray(x_b[b, hq * LQS:(hq + 1) * LQS, :]),
            "ctx": np.ascontiguousarray(ctx_b[b]),
            "mask01": np.ascontiguousarray(mask01[b]),
            "wq": wq_f,
            "wk": wk_f,
            "wv": wv_f,
            "wo": wo_f,
        }
        if has_bias:
            m["cq"], m["ck"], m["cv"] = cq, ck, cv
        in_maps.append(m)
    return in_maps, has_bias


def kernel(x, context, key_padding_mask, ln_q_w, ln_q_b, ln_ctx_w, ln_ctx_b,
           Wq, Wk, Wv, Wo, tau, _trace=False):
    in_maps, has_bias = make_core_inputs(
        x, context, key_padding_mask, ln_q_w, ln_q_b, ln_ctx_w, ln_ctx_b,
        Wq, Wk, Wv, Wo, tau,
    )
    inv_tau = 1.0 / (float(np.asarray(tau)) + 1e-6)
    nc = _get_program(inv_tau, has_bias)
    res = run_bass_kernel_spmd(nc, in_maps, list(range(8)), trace=_trace)
    out = np.empty((B, LQ, DQ), np.float32)
    for core in range(8):
        b, hq = core // 2, core % 2
        out[b, hq * LQS:(hq + 1) * LQS, :] = res.results[core]["out"]
    if _trace:
        return out, res
    return out


# revision 22
# speedup vs baseline: 1.0098x; 1.0098x over previous
"""Trainium2 Bass kernel for nn_CrossAttnBlock (B=4, Lq=Lk=2048, D=1024, H=16, Dh=64).

Sharding: 8 cores = (batch b in 0..3) x (query half in 0..1). Each core computes
cross-attention for 1024 query tokens of one batch against that batch's full
2048-token context. K/V work is duplicated across the two cores of a batch
(zero collectives needed).

All matmul operands are bf16 (PSUM accumulates fp32); this hardware duty-cycle
throttles the PE to ~50% under sustained load, so the program minimizes PE
cycles and keeps the PE stream gap-free:
  - token->feature transposes on the DMA crossbar (dma_start_transpose): the
    PE does zero transposes and the DVE does zero PSUM evictions (ACT evicts)
  - LN -> projection loops are software-pipelined in stages (A: load+LN+xbar,
    B: matmuls+evict, C: l2norm+xbar) with A emitted two tiles ahead of B, so
    the in-order DVE/ACT queues never serialize against the PE (each
    cross-engine hop costs ~1us of latency)
  - attention S/O software-pipelined: O(t-1) emitted after S(t) so the PE
    never waits on the ACT exp; per-head softmax normalization (DVE reciprocal
    + ones-matmul partition broadcast + multiply) is deferred into the middle
    of the next head's loop so its PE matmul never stalls the stream
  - weights and the first ctx tiles prefetched across phase boundaries

rsqrt is computed as exp(-0.5*ln(s)) (+ one Newton step for the l2 norms; the
LN rsqrt skips Newton since a per-token scale error cancels in the normalized
q-hat/k-hat). Softmax needs no max-subtraction: |scores/tau'| <= 8.01.
"""

import contextlib

import numpy as np

import bass_rust
import concourse.bass as bass
import concourse.tile as tile
from concourse import mybir
from concourse.bass_utils import run_bass_kernel_spmd

F32 = mybir.dt.float32
F32R = mybir.dt.float32r
BF16 = mybir.dt.bfloat16
AF = mybir.ActivationFunctionType
ALU = mybir.AluOpType

B, LQ, LK = 4, 2048, 2048
DQ, DC = 1024, 1024
H, DH = 16, 64
INNER = H * DH
LN_EPS = 1e-5

LQS = LQ // 2          # query tokens per core (1024)
NQT = LQS // 128       # 8 query token tiles
NKT = LK // 128        # 16 k token tiles
NF = DQ // 128         # 8 feature tiles


def _rsqrt(nc, pool, s_ap, out_ap, newton=True):
    """out = 1/sqrt(s) via exp(-0.5*ln(s)), optionally one Newton step."""
    p, n = s_ap.shape[0], s_ap.free_size()
    t = pool.tile([p, n], F32, tag="rsq_t")
    nc.scalar.activation(out=t[:, :n], in_=s_ap, func=AF.Ln)
    nc.scalar.activation(out=out_ap, in_=t[:, :n], func=AF.Exp, scale=-0.5)
    if newton:
        # r = r * (1.5 - 0.5 * s * r^2)
        a = pool.tile([p, n], F32, tag="rsq_a")
        nc.vector.tensor_mul(out=a[:, :n], in0=out_ap, in1=out_ap)
        nc.vector.tensor_mul(out=a[:, :n], in0=a[:, :n], in1=s_ap)
        nc.vector.tensor_scalar(
            out=a[:, :n], in0=a[:, :n], scalar1=-0.5, scalar2=1.5,
            op0=ALU.mult, op1=ALU.add,
        )
        nc.vector.tensor_mul(out=out_ap, in0=out_ap, in1=a[:, :n])


def _layernorm_tile(nc, pool, x_ap, z_ap):
    """z = (x - mean) * rsqrt(var + eps) for a [128, 1024] token-major tile.

    No Newton step on the rsqrt: the resulting per-token scale error is
    common-mode and cancels exactly in the l2-normalized q-hat/k-hat; only v
    inherits it (~1e-3), far inside the error budget. Keeps the serial
    cross-engine chain short (each hop costs ~1us of latency)."""
    p = x_ap.shape[0]
    stats = pool.tile([p, 2, 6], F32, tag="ln_stats")
    x3 = x_ap.rearrange("p (c f) -> p c f", c=2)
    for c in range(2):
        nc.vector.bn_stats(out=stats[:, c, :], in_=x3[:, c, :])
    mv = pool.tile([p, 2], F32, tag="ln_mv")
    nc.vector.bn_aggr(out=mv[:], in_=stats[:])
    s = pool.tile([p, 1], F32, tag="ln_s")
    nc.vector.tensor_scalar(
        out=s[:], in0=mv[:, 1:2], scalar1=LN_EPS, scalar2=None, op0=ALU.add,
    )
    inv = pool.tile([p, 1], F32, tag="ln_inv")
    _rsqrt(nc, pool, s[:], inv[:], newton=False)
    nc.vector.tensor_scalar(
        out=z_ap, in0=x_ap, scalar1=mv[:, 0:1], scalar2=inv[:],
        op0=ALU.subtract, op1=ALU.mult,
    )


def build_program(inv_tau: float, has_bias: bool):
    """Build the single-core SPMD bass program."""
    nc = bass.Bass()

    xs = nc.declare_dram_parameter("xs", [LQS, DQ], BF16, isOutput=False)
    ctx = nc.declare_dram_parameter("ctx", [LK, DC], BF16, isOutput=False)
    mask01 = nc.declare_dram_parameter("mask01", [LK], F32, isOutput=False)
    wq = nc.declare_dram_parameter("wq", [DQ, INNER], BF16, isOutput=False)
    wk = nc.declare_dram_parameter("wk", [DC, INNER], BF16, isOutput=False)
    wv = nc.declare_dram_parameter("wv", [DC, INNER], BF16, isOutput=False)
    wo = nc.declare_dram_parameter("wo", [INNER, DQ], BF16, isOutput=False)
    if has_bias:
        cq = nc.declare_dram_parameter("cq", [INNER], F32, isOutput=False)
        ck = nc.declare_dram_parameter("ck", [INNER], F32, isOutput=False)
        cv = nc.declare_dram_parameter("cv", [INNER], F32, isOutput=False)
    out = nc.declare_dram_parameter("out", [LQS, DQ], F32, isOutput=True)

    with tile.TileContext(nc) as tc:
        with contextlib.ExitStack() as stack:
            consts = stack.enter_context(tc.tile_pool(name="consts", bufs=1))
            ones_f = consts.tile([1, DH], F32)
            nc.vector.memset(ones_f[:], 1.0)
            ones_r = consts.tile([1, DH], F32R)
            nc.vector.tensor_copy(out=ones_r[:], in_=ones_f[:])
            ones_dh = ones_r[:]

            # mask as [128, NKT] float (token t*128+p at [p, t])
            mask_sb = consts.tile([128, NKT], F32)
            nc.sync.dma_start(
                out=mask_sb[:], in_=mask01.rearrange("(t p) -> p t", p=128)
            )
            if has_bias:
                cq_b = consts.tile([128, INNER], F32)
                ck_b = consts.tile([128, INNER], F32)
                cv_b = consts.tile([128, INNER], F32)
                for dst, src in ((cq_b, cq), (ck_b, ck), (cv_b, cv)):
                    bc = bass.AP(
                        tensor=src.tensor, offset=src.offset,
                        ap=[[0, 128]] + list(src.ap),
                    )
                    nc.gpsimd.dma_start(out=dst[:], in_=bc)

            small = stack.enter_context(tc.tile_pool(name="small", bufs=2))

            # persistent feature-major activations (bf16)
            persist = stack.enter_context(tc.tile_pool(name="persist", bufs=1))
            qhatT = persist.tile([128, NF, LQS], BF16, name="qhatT")
            oaT = persist.tile([128, NF, LQS], BF16, name="oaT")

            # kT/vpp persist into P2b (entered before kv era: LIFO order)
            kvw_pool = stack.enter_context(tc.tile_pool(name="kvw", bufs=1))
            kT = kvw_pool.tile([128, NF, LK], BF16, name="kT")
            vpp = kvw_pool.tile([128, NKT, H, DH + 1], BF16, name="vpp")
            # preset all mask columns once: vpp[:, t, h, 64] = mask[:, t]
            nc.gpsimd.tensor_copy(
                out=vpp[:, :, :, DH:DH + 1],
                in_=mask_sb[:].unsqueeze(2).broadcast_to([128, NKT, H]
                                                        ).unsqueeze(3),
            )

            with contextlib.ExitStack() as kv_era:
                kvin_pool = kv_era.enter_context(
                    tc.tile_pool(name="kvin", bufs=1)
                )
                kvT = kvin_pool.tile([128, NF, LK], BF16, name="kvT")
                ln_pool = kv_era.enter_context(
                    tc.tile_pool(name="ln", bufs=3)
                )

                def ctx_stage_a(t):
                    xt = ln_pool.tile([128, DQ], BF16, tag="ln_x",
                                      name=f"kx{t}")
                    nc.sync.dma_start(
                        out=xt[:], in_=ctx[t * 128:(t + 1) * 128, :]
                    )
                    zt = ln_pool.tile([128, DQ], BF16, tag="ln_z",
                                      name=f"kz{t}")
                    _layernorm_tile(nc, small, xt[:], zt[:])
                    nc.sync.dma_start_transpose(
                        out=kvT[:, :, t * 128:(t + 1) * 128], in_=zt[:]
                    )

                # ---- P1: x LayerNorm + q projection ----
                # Software-pipelined stages: A(t)=load+LN+xbar, B(t)=matmuls+
                # evict, C(t)=l2norm+xbar. Emitting A(t) before B(t-1) before
                # C(t-2) keeps the in-order engine queues from serializing the
                # per-tile cross-engine chains against the PE.
                with contextlib.ExitStack() as p1:
                    wq_pool = p1.enter_context(tc.tile_pool(name="wq", bufs=1))
                    wq_sb = wq_pool.tile([128, NF, INNER], BF16)
                    qin_pool = p1.enter_context(tc.tile_pool(name="qin", bufs=1))
                    qinT = qin_pool.tile([128, NF, LQS], BF16, name="qinT")
                    mm_psum = p1.enter_context(
                        tc.tile_pool(name="mm_psum1", bufs=2, space="PSUM")
                    )
                    qt_pool = p1.enter_context(tc.tile_pool(name="qtok", bufs=3))
                    qtok_of = {}

                    def q_stage_a(t):
                        xt = ln_pool.tile([128, DQ], BF16, tag="ln_x",
                                          name=f"qx{t}")
                        nc.sync.dma_start(
                            out=xt[:], in_=xs[t * 128:(t + 1) * 128, :]
                        )
                        zt = ln_pool.tile([128, DQ], BF16, tag="ln_z",
                                          name=f"qz{t}")
                        _layernorm_tile(nc, small, xt[:], zt[:])
                        nc.sync.dma_start_transpose(
                            out=qinT[:, :, t * 128:(t + 1) * 128], in_=zt[:]
                        )

                    def q_stage_b(t):
                        qtok = qt_pool.tile([128, INNER], BF16, tag="qtok",
                                            name=f"qtok{t}")
                        qtok_of[t] = qtok
                        ps = mm_psum.tile([128, INNER], F32, tag="mm",
                                          name=f"qps{t}")
                        for n in range(2):
                            for fi in range(NF):
                                nc.tensor.matmul(
                                    ps[:, n * 512:(n + 1) * 512],
                                    qinT[:, fi, t * 128:(t + 1) * 128],
                                    wq_sb[:, fi, n * 512:(n + 1) * 512],
                                    start=(fi == 0),
                                    stop=(fi == NF - 1),
                                )
                        if has_bias:
                            nc.vector.tensor_add(
                                out=qtok[:], in0=ps[:], in1=cq_b[:]
                            )
                        else:
                            nc.scalar.copy(out=qtok[:], in_=ps[:])

                    def q_stage_c(t):
                        qtok = qtok_of.pop(t)
                        sq = qt_pool.tile([128, INNER], BF16, tag="qsq",
                                          name=f"qsq{t}")
                        nc.vector.tensor_mul(
                            out=sq[:], in0=qtok[:], in1=qtok[:]
                        )
                        ssq = small.tile([128, H], F32, tag="qssq")
                        nc.vector.tensor_reduce(
                            out=ssq[:],
                            in_=sq[:].rearrange("p (h d) -> p h d", h=H),
                            axis=mybir.AxisListType.X,
                            op=ALU.add,
                        )
                        rq = small.tile([128, H], F32, tag="qrq")
                        _rsqrt(nc, small, ssq[:], rq[:])
                        q3 = qtok[:].rearrange("p (h d) -> p h d", h=H)
                        nc.vector.tensor_tensor(
                            out=q3,
                            in0=q3,
                            in1=rq[:].unsqueeze(2).broadcast_to([128, H, DH]),
                            op=ALU.mult,
                        )
                        nc.sync.dma_start_transpose(
                            out=qhatT[:, :, t * 128:(t + 1) * 128], in_=qtok[:]
                        )

                    # x0's LN starts before the wq weight load so the DMA
                    # queue delivers the first tile immediately.
                    q_stage_a(0)
                    for fi in range(NF):
                        nc.sync.dma_start(
                            out=wq_sb[:, fi, :],
                            in_=wq[fi * 128:(fi + 1) * 128, :],
                        )
                    for s in range(1, NQT + 3):
                        if s < NQT:
                            q_stage_a(s)
                        if s == NQT:
                            ctx_stage_a(0)
                        if s == NQT + 1:
                            ctx_stage_a(1)
                        if 2 <= s < NQT + 2:
                            q_stage_b(s - 2)
                        if s >= 3:
                            q_stage_c(s - 3)

                # ---- P2a: ctx LayerNorm + k/v projection, merged per tile ----
                # kT[:, fi, :] holds features [fi*128, (fi+1)*128) = heads
                # (2*fi, 2*fi+1) stacked 64+64 on partitions.
                with contextlib.ExitStack() as p2a:
                    wkv_pool = p2a.enter_context(
                        tc.tile_pool(name="wkv", bufs=1)
                    )
                    wk_sb = wkv_pool.tile([128, NF, INNER], BF16, name="wk_sb")
                    wv_sb = wkv_pool.tile([128, NF, INNER], BF16, name="wv_sb")
                    for fi in range(NF):
                        nc.sync.dma_start(
                            out=wk_sb[:, fi, :],
                            in_=wk[fi * 128:(fi + 1) * 128, :],
                        )
                        nc.sync.dma_start(
                            out=wv_sb[:, fi, :],
                            in_=wv[fi * 128:(fi + 1) * 128, :],
                        )
                    ln_pool = p2a.enter_context(tc.tile_pool(name="ln2", bufs=3))
                    mm_psum = p2a.enter_context(
                        tc.tile_pool(name="mm_psum2", bufs=2, space="PSUM")
                    )
                    kt_pool = p2a.enter_context(
                        tc.tile_pool(name="ktok", bufs=3)
                    )
                    ktok_of = {}
                    ps_v_of = {}

                    def k_stage_b(t):
                        ps_k = mm_psum.tile([128, INNER], F32, tag="mm_k",
                                            name=f"psk{t}")
                        ps_v = mm_psum.tile([128, INNER], F32, tag="mm_v",
                                            name=f"psv{t}")
                        ps_v_of[t] = ps_v
                        for n in range(2):
                            for fi in range(NF):
                                nc.tensor.matmul(
                                    ps_k[:, n * 512:(n + 1) * 512],
                                    kvT[:, fi, t * 128:(t + 1) * 128],
                                    wk_sb[:, fi, n * 512:(n + 1) * 512],
                                    start=(fi == 0), stop=(fi == NF - 1),
                                )
                        ktok = kt_pool.tile([128, INNER], BF16, tag="ktok",
                                            name=f"ktok{t}")
                        ktok_of[t] = ktok
                        if has_bias:
                            nc.vector.tensor_add(
                                out=ktok[:], in0=ps_k[:], in1=ck_b[:]
                            )
                        else:
                            nc.scalar.copy(out=ktok[:], in_=ps_k[:])
                        for n in range(2):
                            for fi in range(NF):
                                nc.tensor.matmul(
                                    ps_v[:, n * 512:(n + 1) * 512],
                                    kvT[:, fi, t * 128:(t + 1) * 128],
                                    wv_sb[:, fi, n * 512:(n + 1) * 512],
                                    start=(fi == 0), stop=(fi == NF - 1),
                                )

                    def k_stage_c(t):
                        ktok = ktok_of.pop(t)
                        ps_v = ps_v_of.pop(t)
                        # V'' = [v * mask | mask] (mask col preset above)
                        mt = mask_sb[:, t:t + 1]
                        if has_bias:
                            vtmp = kt_pool.tile([128, INNER], F32, tag="vtmp",
                                                name=f"vtmp{t}")
                            nc.vector.tensor_add(
                                out=vtmp[:], in0=ps_v[:], in1=cv_b[:]
                            )
                            vsrc = vtmp[:]
                        else:
                            vsrc = ps_v[:]
                        nc.vector.tensor_scalar_mul(
                            out=vpp[:, t, :, 0:DH],
                            in0=vsrc.rearrange("p (h d) -> p h d", h=H),
                            scalar1=mt,
                        )
                        # k l2 norm (in place) then xbar transpose
                        sqk = kt_pool.tile([128, INNER], BF16, tag="ksq",
                                           name=f"ksq{t}")
                        nc.vector.tensor_mul(
                            out=sqk[:], in0=ktok[:], in1=ktok[:]
                        )
                        ssqk = small.tile([128, H], F32, tag="kssq")
                        nc.vector.tensor_reduce(
                            out=ssqk[:],
                            in_=sqk[:].rearrange("p (h d) -> p h d", h=H),
                            axis=mybir.AxisListType.X,
                            op=ALU.add,
                        )
                        rk = small.tile([128, H], F32, tag="krk")
                        _rsqrt(nc, small, ssqk[:], rk[:])
                        k3 = ktok[:].rearrange("p (h d) -> p h d", h=H)
                        nc.vector.tensor_tensor(
                            out=k3,
                            in0=k3,
                            in1=rk[:].unsqueeze(2).broadcast_to([128, H, DH]),
                            op=ALU.mult,
                        )
                        nc.sync.dma_start_transpose(
                            out=kT[:, :, t * 128:(t + 1) * 128], in_=ktok[:]
                        )

                    for s in range(2, NKT + 3):
                        if s < NKT:
                            ctx_stage_a(s)
                        if s < NKT + 2:
                            k_stage_b(s - 2)
                        if s >= 3:
                            k_stage_c(s - 3)

            # ---- P2b: attention per head, S/O software-pipelined ----
            wo_pool = stack.enter_context(tc.tile_pool(name="wo", bufs=1))
            wo_sb = wo_pool.tile([128, NF, DQ], BF16)
            for fi in range(NF):
                nc.sync.dma_start(
                    out=wo_sb[:, fi, :], in_=wo[fi * 128:(fi + 1) * 128, :]
                )
            with contextlib.ExitStack() as p2b:
                exp_pool = p2b.enter_context(tc.tile_pool(name="exp", bufs=4))
                rec_pool = p2b.enter_context(tc.tile_pool(name="rec", bufs=2))
                s_psum = p2b.enter_context(
                    tc.tile_pool(name="s_psum", bufs=2, space="PSUM")
                )
                ps_o_pool = p2b.enter_context(
                    tc.tile_pool(name="ps_o", bufs=2, space="PSUM")
                )
                ps_o_of = {}
                es_of = {}

                def emit_normalize(g):
                    """Denominator reciprocal + partition-broadcast + scale."""
                    gfi, gr = g // 2, (g % 2) * DH
                    ps_o = ps_o_of.pop(g)
                    rec_f = rec_pool.tile([1, LQS], F32, tag="rec_f")
                    nc.vector.reciprocal(out=rec_f[:], in_=ps_o[DH:DH + 1, :])
                    recr = rec_pool.tile([1, LQS], F32R, tag="recr")
                    nc.vector.tensor_copy(out=recr[:], in_=rec_f[:])
                    rb_ps = s_psum.tile([DH, LQS], F32, tag="mm_s")
                    for c in range(2):
                        nc.tensor.matmul(
                            rb_ps[:, c * 512:(c + 1) * 512],
                            ones_dh,
                            recr[0:1, c * 512:(c + 1) * 512],
                            start=True,
                            stop=True,
                        )
                    recb = rec_pool.tile([DH, LQS], F32, tag="recb")
                    nc.vector.tensor_copy(out=recb[:], in_=rb_ps[:])
                    nc.vector.tensor_tensor(
                        out=oaT[gr:gr + DH, gfi, :],
                        in0=ps_o[0:DH, :],
                        in1=recb[:],
                        op=ALU.mult,
                    )

                def emit_o(g, t):
                    gfi, gr = g // 2, (g % 2) * DH
                    for c in range(2):
                        nc.tensor.matmul(
                            ps_o_of[g][:, c * 512:(c + 1) * 512],
                            vpp[:, t, g, :],
                            es_of[(g, t)][:, c * 512:(c + 1) * 512],
                            start=(t == 0),
                            stop=(t == NKT - 1),
                        )
                    del es_of[(g, t)]

                for g in range(H):
                    gfi, gr = g // 2, (g % 2) * DH
                    ps_o_of[g] = ps_o_pool.tile(
                        [DH + 1, LQS], F32, tag="ps_o", name=f"ps_o{g}"
                    )
                    for t in range(NKT):
                        ps_s = s_psum.tile([128, LQS], F32, tag="mm_s")
                        for c in range(2):
                            nc.tensor.matmul(
                                ps_s[:, c * 512:(c + 1) * 512],
                                kT[gr:gr + DH, gfi, t * 128:(t + 1) * 128],
                                qhatT[gr:gr + DH, gfi, c * 512:(c + 1) * 512],
                                start=True,
                                stop=True,
                            )
                        es = exp_pool.tile([128, LQS], BF16, tag="es")
                        nc.scalar.activation(
                            out=es[:], in_=ps_s[:], func=AF.Exp, scale=inv_tau,
                        )
                        es_of[(g, t)] = es
                        if t >= 1:
                            emit_o(g, t - 1)
                        if t == 6 and g >= 1:
                            emit_normalize(g - 1)
                    emit_o(g, NKT - 1)
                emit_normalize(H - 1)

            # ---- P3: output projection ----
            with contextlib.ExitStack() as p3:
                mm_psum = p3.enter_context(
                    tc.tile_pool(name="mm_psum3", bufs=3, space="PSUM")
                )
                fin_pool = p3.enter_context(tc.tile_pool(name="fin", bufs=3))
                for t in range(NQT):
                    ft = fin_pool.tile([128, DQ], F32, tag="fin")
                    for n in range(2):
                        ps = mm_psum.tile([128, 512], F32, tag="mm")
                        for fi in range(NF):
                            nc.tensor.matmul(
                                ps[:],
                                oaT[:, fi, t * 128:(t + 1) * 128],
                                wo_sb[:, fi, n * 512:(n + 1) * 512],
                                start=(fi == 0),
                                stop=(fi == NF - 1),
                            )
                        nc.scalar.copy(
                            out=ft[:, n * 512:(n + 1) * 512], in_=ps[:]
                        )
                    nc.sync.dma_start(
                        out=out[t * 128:(t + 1) * 128, :], in_=ft[:]
                    )

    return nc


def split_multi_waits(nc):
    """walrus in this environment rejects >1 sync wait per instruction; move
    extras onto same-engine NOPs immediately preceding the instruction."""
    ctr = 0
    for f in nc.m.functions:
        for bb in f.blocks:
            new = []
            for inst in bb.instructions:
                si = inst.sync_info
                if si is not None and len(si.on_wait) > 1:
                    waits = list(si.on_wait)
                    for w in waits[:-1]:
                        nop = bass_rust.InstNoOp(name=f"I-wsplit-{ctr}")
                        ctr += 1
                        nop.engine = inst.engine
                        nop.sync_info = bass_rust.SyncInfo(
                            on_wait=[w], on_update=[]
                        )
                        new.append(nop)
                    inst.sync_info = bass_rust.SyncInfo(
                        on_wait=[waits[-1]], on_update=list(si.on_update)
                    )
                new.append(inst)
            bb.instructions[:] = new
    return ctr


_PROGRAM_CACHE = {}


def _get_program(inv_tau: float, has_bias: bool):
    key = (round(float(inv_tau), 12), has_bias)
    if key not in _PROGRAM_CACHE:
        nc = build_program(float(inv_tau), has_bias)
        split_multi_waits(nc)
        _PROGRAM_CACHE[key] = nc
    return _PROGRAM_CACHE[key]


def make_core_inputs(x, context, key_padding_mask, ln_q_w, ln_q_b, ln_ctx_w,
                     ln_ctx_b, Wq, Wk, Wv, Wo, tau):
    """Shard + host-side weight folding. Returns (in_maps, has_bias)."""
    import ml_dtypes

    f32 = np.float32
    bf16 = ml_dtypes.bfloat16
    x = np.asarray(x, f32)
    context = np.asarray(context, f32)
    mask01 = 1.0 - np.asarray(key_padding_mask).astype(f32)  # 1 = keep
    ln_q_w = np.asarray(ln_q_w, f32)
    ln_q_b = np.asarray(ln_q_b, f32)
    ln_ctx_w = np.asarray(ln_ctx_w, f32)
    ln_ctx_b = np.asarray(ln_ctx_b, f32)
    Wq = np.asarray(Wq, f32)
    Wk = np.asarray(Wk, f32)
    Wv = np.asarray(Wv, f32)
    Wo = np.asarray(Wo, f32)

    # fold LN affine into projections: (z*w + b) @ W = z @ (w*W) + b@W
    wq_f = np.ascontiguousarray(Wq * ln_q_w[:, None]).astype(bf16)
    wk_f = np.ascontiguousarray(Wk * ln_ctx_w[:, None]).astype(bf16)
    wv_f = np.ascontiguousarray(Wv * ln_ctx_w[:, None]).astype(bf16)
    wo_f = Wo.astype(bf16)
    has_bias = bool(np.any(ln_q_b != 0.0) or np.any(ln_ctx_b != 0.0))
    cq = (ln_q_b @ Wq).astype(f32)
    ck = (ln_ctx_b @ Wk).astype(f32)
    cv = (ln_ctx_b @ Wv).astype(f32)

    x_b = x.astype(bf16)
    ctx_b = context.astype(bf16)

    in_maps = []
    for core in range(8):
        b, hq = core // 2, core % 2
        m = {
            "xs": np.ascontiguousarray(x_b[b, hq * LQS:(hq + 1) * LQS, :]),
            "ctx": np.ascontiguousarray(ctx_b[b]),
            "mask01": np.ascontiguousarray(mask01[b]),
            "wq": wq_f,
            "wk": wk_f,
            "wv": wv_f,
            "wo": wo_f,
        }
        if has_bias:
            m["cq"], m["ck"], m["cv"] = cq, ck, cv
        in_maps.append(m)
    return in_maps, has_bias


def kernel(x, context, key_padding_mask, ln_q_w, ln_q_b, ln_ctx_w, ln_ctx_b,
           Wq, Wk, Wv, Wo, tau, _trace=False):
    in_maps, has_bias = make_core_inputs(
        x, context, key_padding_mask, ln_q_w, ln_q_b, ln_ctx_w, ln_ctx_b,
        Wq, Wk, Wv, Wo, tau,
    )
    inv_tau = 1.0 / (float(np.asarray(tau)) + 1e-6)
    nc = _get_program(inv_tau, has_bias)
    res = run_bass_kernel_spmd(nc, in_maps, list(range(8)), trace=_trace)
    out = np.empty((B, LQ, DQ), np.float32)
    for core in range(8):
        b, hq = core // 2, core % 2
        out[b, hq * LQS:(hq + 1) * LQS, :] = res.results[core]["out"]
    if _trace:
        return out, res
    return out


# revision 24
# speedup vs baseline: 1.3268x; 1.3139x over previous
"""Trainium2 Bass kernel for nn_CrossAttnBlock (B=4, Lq=Lk=2048, D=1024, H=16, Dh=64).

Sharding: 8 cores = (batch b in 0..3) x (query half in 0..1). Each core computes
cross-attention for 1024 query tokens of one batch against that batch's full
2048-token context. K/V work is duplicated across the two cores of a batch
(zero collectives needed).

All matmul operands are bf16 (PSUM accumulates fp32); this hardware duty-cycle
throttles the PE to ~50% under sustained load, so the program minimizes PE
cycles and keeps the PE stream gap-free:
  - token->feature transposes on the DMA crossbar (dma_start_transpose): the
    PE does zero transposes and the DVE does zero PSUM evictions (ACT evicts)
  - LN -> projection loops are software-pipelined in stages (A: load+LN+xbar,
    B: matmuls+evict, C: l2norm+xbar) with A emitted two tiles ahead of B, so
    the in-order DVE/ACT queues never serialize against the PE (each
    cross-engine hop costs ~1us of latency)
  - attention S/O software-pipelined: O(t-1) emitted after S(t) so the PE
    never waits on the ACT exp; per-head softmax normalization (DVE reciprocal
    + ones-matmul partition broadcast + multiply) is deferred into the middle
    of the next head's loop so its PE matmul never stalls the stream
  - weights and the first ctx tiles prefetched across phase boundaries

rsqrt is computed as exp(-0.5*ln(s)) (+ one Newton step for the l2 norms; the
LN rsqrt skips Newton since a per-token scale error cancels in the normalized
q-hat/k-hat). Softmax needs no max-subtraction: |scores/tau'| <= 8.01.
"""

import contextlib

import numpy as np

import bass_rust
import concourse.bass as bass
import concourse.tile as tile
from concourse import mybir
from concourse.bass_utils import run_bass_kernel_spmd

F32 = mybir.dt.float32
F32R = mybir.dt.float32r
BF16 = mybir.dt.bfloat16
AF = mybir.ActivationFunctionType
ALU = mybir.AluOpType

B, LQ, LK = 4, 2048, 2048
DQ, DC = 1024, 1024
H, DH = 16, 64
INNER = H * DH
LN_EPS = 1e-5

LQS = LQ // 2          # query tokens per core (1024)
NQT = LQS // 128       # 8 query token tiles
NKT = LK // 128        # 16 k token tiles
NF = DQ // 128         # 8 feature tiles


def _rsqrt(nc, pool, s_ap, out_ap, newton=True):
    """out = 1/sqrt(s) via exp(-0.5*ln(s)), optionally one Newton step."""
    p, n = s_ap.shape[0], s_ap.free_size()
    t = pool.tile([p, n], F32, tag="rsq_t")
    nc.scalar.activation(out=t[:, :n], in_=s_ap, func=AF.Ln)
    nc.scalar.activation(out=out_ap, in_=t[:, :n], func=AF.Exp, scale=-0.5)
    if newton:
        # r = r * (1.5 - 0.5 * s * r^2)
        a = pool.tile([p, n], F32, tag="rsq_a")
        nc.vector.tensor_mul(out=a[:, :n], in0=out_ap, in1=out_ap)
        nc.vector.tensor_mul(out=a[:, :n], in0=a[:, :n], in1=s_ap)
        nc.vector.tensor_scalar(
            out=a[:, :n], in0=a[:, :n], scalar1=-0.5, scalar2=1.5,
            op0=ALU.mult, op1=ALU.add,
        )
        nc.vector.tensor_mul(out=out_ap, in0=out_ap, in1=a[:, :n])


def _layernorm_tile(nc, pool, x_ap, z_ap):
    """z = (x - mean) * rsqrt(var + eps) for a [128, 1024] token-major tile.

    No Newton step on the rsqrt: the resulting per-token scale error is
    common-mode and cancels exactly in the l2-normalized q-hat/k-hat; only v
    inherits it (~1e-3), far inside the error budget. Keeps the serial
    cross-engine chain short (each hop costs ~1us of latency)."""
    p = x_ap.shape[0]
    stats = pool.tile([p, 2, 6], F32, tag="ln_stats")
    x3 = x_ap.rearrange("p (c f) -> p c f", c=2)
    for c in range(2):
        nc.vector.bn_stats(out=stats[:, c, :], in_=x3[:, c, :])
    mv = pool.tile([p, 2], F32, tag="ln_mv")
    nc.vector.bn_aggr(out=mv[:], in_=stats[:])
    s = pool.tile([p, 1], F32, tag="ln_s")
    nc.vector.tensor_scalar(
        out=s[:], in0=mv[:, 1:2], scalar1=LN_EPS, scalar2=None, op0=ALU.add,
    )
    inv = pool.tile([p, 1], F32, tag="ln_inv")
    _rsqrt(nc, pool, s[:], inv[:], newton=False)
    nc.vector.tensor_scalar(
        out=z_ap, in0=x_ap, scalar1=mv[:, 0:1], scalar2=inv[:],
        op0=ALU.subtract, op1=ALU.mult,
    )


def build_program(inv_tau: float, has_bias: bool):
    """Build the single-core SPMD bass program."""
    nc = bass.Bass()

    xs = nc.declare_dram_parameter("xs", [LQS, DQ], BF16, isOutput=False)
    ctx = nc.declare_dram_parameter("ctx", [LK, DC], BF16, isOutput=False)
    mask01 = nc.declare_dram_parameter("mask01", [LK], F32, isOutput=False)
    wq = nc.declare_dram_parameter("wq", [DQ, INNER], BF16, isOutput=False)
    wk = nc.declare_dram_parameter("wk", [DC, INNER], BF16, isOutput=False)
    wv = nc.declare_dram_parameter("wv", [DC, INNER], BF16, isOutput=False)
    wo = nc.declare_dram_parameter("wo", [INNER, DQ], BF16, isOutput=False)
    if has_bias:
        cq = nc.declare_dram_parameter("cq", [INNER], F32, isOutput=False)
        ck = nc.declare_dram_parameter("ck", [INNER], F32, isOutput=False)
        cv = nc.declare_dram_parameter("cv", [INNER], F32, isOutput=False)
    out = nc.declare_dram_parameter("out", [LQS, DQ], F32, isOutput=True)

    with tile.TileContext(nc) as tc:
        with contextlib.ExitStack() as stack:
            consts = stack.enter_context(tc.tile_pool(name="consts", bufs=1))
            ones_f = consts.tile([1, DH], F32)
            nc.vector.memset(ones_f[:], 1.0)
            ones_r = consts.tile([1, DH], F32R)
            nc.vector.tensor_copy(out=ones_r[:], in_=ones_f[:])
            ones_dh = ones_r[:]

            # mask as [128, NKT] float (token t*128+p at [p, t])
            mask_sb = consts.tile([128, NKT], F32)
            nc.sync.dma_start(
                out=mask_sb[:], in_=mask01.rearrange("(t p) -> p t", p=128)
            )
            if has_bias:
                cq_b = consts.tile([128, INNER], F32)
                ck_b = consts.tile([128, INNER], F32)
                cv_b = consts.tile([128, INNER], F32)
                for dst, src in ((cq_b, cq), (ck_b, ck), (cv_b, cv)):
                    bc = bass.AP(
                        tensor=src.tensor, offset=src.offset,
                        ap=[[0, 128]] + list(src.ap),
                    )
                    nc.gpsimd.dma_start(out=dst[:], in_=bc)

            small = stack.enter_context(tc.tile_pool(name="small", bufs=2))

            # persistent feature-major activations (bf16)
            persist = stack.enter_context(tc.tile_pool(name="persist", bufs=1))
            qhatT = persist.tile([128, NF, LQS], BF16, name="qhatT")
            oaT = persist.tile([128, NF, LQS], BF16, name="oaT")

            # kT/vpp persist into P2b (entered before kv era: LIFO order)
            kvw_pool = stack.enter_context(tc.tile_pool(name="kvw", bufs=1))
            kT = kvw_pool.tile([128, NF, LK], BF16, name="kT")
            vpp = kvw_pool.tile([128, NKT, H, DH + 1], BF16, name="vpp")
            # preset all mask columns once: vpp[:, t, h, 64] = mask[:, t]
            nc.gpsimd.tensor_copy(
                out=vpp[:, :, :, DH:DH + 1],
                in_=mask_sb[:].unsqueeze(2).broadcast_to([128, NKT, H]
                                                        ).unsqueeze(3),
            )

            with contextlib.ExitStack() as kv_era:
                kvin_pool = kv_era.enter_context(
                    tc.tile_pool(name="kvin", bufs=1)
                )
                kvT = kvin_pool.tile([128, NF, LK], BF16, name="kvT")
                ln_pool = kv_era.enter_context(
                    tc.tile_pool(name="ln", bufs=3)
                )

                def ctx_stage_a(t):
                    xt = ln_pool.tile([128, DQ], BF16, tag="ln_x",
                                      name=f"kx{t}")
                    nc.sync.dma_start(
                        out=xt[:], in_=ctx[t * 128:(t + 1) * 128, :]
                    )
                    zt = ln_pool.tile([128, DQ], BF16, tag="ln_z",
                                      name=f"kz{t}")
                    _layernorm_tile(nc, small, xt[:], zt[:])
                    nc.sync.dma_start_transpose(
                        out=kvT[:, :, t * 128:(t + 1) * 128], in_=zt[:]
                    )

                # ---- P1: x LayerNorm + q projection ----
                # Software-pipelined stages: A(t)=load+LN+xbar, B(t)=matmuls+
                # evict, C(t)=l2norm+xbar. Emitting A(t) before B(t-1) before
                # C(t-2) keeps the in-order engine queues from serializing the
                # per-tile cross-engine chains against the PE.
                with contextlib.ExitStack() as p1:
                    wq_pool = p1.enter_context(tc.tile_pool(name="wq", bufs=1))
                    wq_sb = wq_pool.tile([128, NF, INNER], BF16)
                    qin_pool = p1.enter_context(tc.tile_pool(name="qin", bufs=1))
                    qinT = qin_pool.tile([128, NF, LQS], BF16, name="qinT")
                    mm_psum = p1.enter_context(
                        tc.tile_pool(name="mm_psum1", bufs=2, space="PSUM")
                    )
                    qt_pool = p1.enter_context(tc.tile_pool(name="qtok", bufs=3))
                    qtok_of = {}

                    def q_stage_a(t):
                        xt = ln_pool.tile([128, DQ], BF16, tag="ln_x",
                                          name=f"qx{t}")
                        nc.sync.dma_start(
                            out=xt[:], in_=xs[t * 128:(t + 1) * 128, :]
                        )
                        zt = ln_pool.tile([128, DQ], BF16, tag="ln_z",
                                          name=f"qz{t}")
                        _layernorm_tile(nc, small, xt[:], zt[:])
                        nc.sync.dma_start_transpose(
                            out=qinT[:, :, t * 128:(t + 1) * 128], in_=zt[:]
                        )

                    def q_stage_b(t):
                        qtok = qt_pool.tile([128, INNER], BF16, tag="qtok",
                                            name=f"qtok{t}")
                        qtok_of[t] = qtok
                        ps = mm_psum.tile([128, INNER], F32, tag="mm",
                                          name=f"qps{t}")
                        for n in range(2):
                            for fi in range(NF):
                                nc.tensor.matmul(
                                    ps[:, n * 512:(n + 1) * 512],
                                    qinT[:, fi, t * 128:(t + 1) * 128],
                                    wq_sb[:, fi, n * 512:(n + 1) * 512],
                                    start=(fi == 0),
                                    stop=(fi == NF - 1),
                                )
                        if has_bias:
                            nc.vector.tensor_add(
                                out=qtok[:], in0=ps[:], in1=cq_b[:]
                            )
                        else:
                            nc.scalar.copy(out=qtok[:], in_=ps[:])

                    def q_stage_c(t):
                        qtok = qtok_of.pop(t)
                        sq = qt_pool.tile([128, INNER], BF16, tag="qsq",
                                          name=f"qsq{t}")
                        nc.vector.tensor_mul(
                            out=sq[:], in0=qtok[:], in1=qtok[:]
                        )
                        ssq = small.tile([128, H], F32, tag="qssq")
                        nc.vector.tensor_reduce(
                            out=ssq[:],
                            in_=sq[:].rearrange("p (h d) -> p h d", h=H),
                            axis=mybir.AxisListType.X,
                            op=ALU.add,
                        )
                        rq = small.tile([128, H], F32, tag="qrq")
                        _rsqrt(nc, small, ssq[:], rq[:])
                        q3 = qtok[:].rearrange("p (h d) -> p h d", h=H)
                        nc.vector.tensor_tensor(
                            out=q3,
                            in0=q3,
                            in1=rq[:].unsqueeze(2).broadcast_to([128, H, DH]),
                            op=ALU.mult,
                        )
                        nc.sync.dma_start_transpose(
                            out=qhatT[:, :, t * 128:(t + 1) * 128], in_=qtok[:]
                        )

                    # x0's LN starts before the wq weight load so the DMA
                    # queue delivers the first tile immediately.
                    q_stage_a(0)
                    for fi in range(NF):
                        nc.sync.dma_start(
                            out=wq_sb[:, fi, :],
                            in_=wq[fi * 128:(fi + 1) * 128, :],
                        )
                    for s in range(1, NQT + 3):
                        if s < NQT:
                            q_stage_a(s)
                        if s == NQT:
                            ctx_stage_a(0)
                        if s == NQT + 1:
                            ctx_stage_a(1)
                        if 2 <= s < NQT + 2:
                            q_stage_b(s - 2)
                        if s >= 3:
                            q_stage_c(s - 3)

                # ---- P2a: ctx LayerNorm + k/v projection, merged per tile ----
                # kT[:, fi, :] holds features [fi*128, (fi+1)*128) = heads
                # (2*fi, 2*fi+1) stacked 64+64 on partitions.
                with contextlib.ExitStack() as p2a:
                    wkv_pool = p2a.enter_context(
                        tc.tile_pool(name="wkv", bufs=1)
                    )
                    wk_sb = wkv_pool.tile([128, NF, INNER], BF16, name="wk_sb")
                    wv_sb = wkv_pool.tile([128, NF, INNER], BF16, name="wv_sb")
                    for fi in range(NF):
                        nc.sync.dma_start(
                            out=wk_sb[:, fi, :],
                            in_=wk[fi * 128:(fi + 1) * 128, :],
                        )
                        nc.sync.dma_start(
                            out=wv_sb[:, fi, :],
                            in_=wv[fi * 128:(fi + 1) * 128, :],
                        )
                    ln_pool = p2a.enter_context(tc.tile_pool(name="ln2", bufs=3))
                    mm_psum = p2a.enter_context(
                        tc.tile_pool(name="mm_psum2", bufs=2, space="PSUM")
                    )
                    kt_pool = p2a.enter_context(
                        tc.tile_pool(name="ktok", bufs=3)
                    )
                    ktok_of = {}
                    ps_v_of = {}

                    def k_stage_b(t):
                        ps_k = mm_psum.tile([128, INNER], F32, tag="mm_k",
                                            name=f"psk{t}")
                        ps_v = mm_psum.tile([128, INNER], F32, tag="mm_v",
                                            name=f"psv{t}")
                        ps_v_of[t] = ps_v
                        for n in range(2):
                            for fi in range(NF):
                                nc.tensor.matmul(
                                    ps_k[:, n * 512:(n + 1) * 512],
                                    kvT[:, fi, t * 128:(t + 1) * 128],
                                    wk_sb[:, fi, n * 512:(n + 1) * 512],
                                    start=(fi == 0), stop=(fi == NF - 1),
                                )
                        ktok = kt_pool.tile([128, INNER], BF16, tag="ktok",
                                            name=f"ktok{t}")
                        ktok_of[t] = ktok
                        if has_bias:
                            nc.vector.tensor_add(
                                out=ktok[:], in0=ps_k[:], in1=ck_b[:]
                            )
                        else:
                            nc.scalar.copy(out=ktok[:], in_=ps_k[:])
                        for n in range(2):
                            for fi in range(NF):
                                nc.tensor.matmul(
                                    ps_v[:, n * 512:(n + 1) * 512],
                                    kvT[:, fi, t * 128:(t + 1) * 128],
                                    wv_sb[:, fi, n * 512:(n + 1) * 512],
                                    start=(fi == 0), stop=(fi == NF - 1),
                                )

                    def k_stage_c(t):
                        ktok = ktok_of.pop(t)
                        ps_v = ps_v_of.pop(t)
                        # V'' = [v * mask | mask] (mask col preset above)
                        mt = mask_sb[:, t:t + 1]
                        if has_bias:
                            vtmp = kt_pool.tile([128, INNER], F32, tag="vtmp",
                                                name=f"vtmp{t}")
                            nc.vector.tensor_add(
                                out=vtmp[:], in0=ps_v[:], in1=cv_b[:]
                            )
                            vsrc = vtmp[:]
                        else:
                            vsrc = ps_v[:]
                        nc.vector.tensor_scalar_mul(
                            out=vpp[:, t, :, 0:DH],
                            in0=vsrc.rearrange("p (h d) -> p h d", h=H),
                            scalar1=mt,
                        )
                        # k l2 norm (in place) then xbar transpose
                        sqk = kt_pool.tile([128, INNER], BF16, tag="ksq",
                                           name=f"ksq{t}")
                        nc.vector.tensor_mul(
                            out=sqk[:], in0=ktok[:], in1=ktok[:]
                        )
                        ssqk = small.tile([128, H], F32, tag="kssq")
                        nc.vector.tensor_reduce(
                            out=ssqk[:],
                            in_=sqk[:].rearrange("p (h d) -> p h d", h=H),
                            axis=mybir.AxisListType.X,
                            op=ALU.add,
                        )
                        rk = small.tile([128, H], F32, tag="krk")
                        _rsqrt(nc, small, ssqk[:], rk[:])
                        k3 = ktok[:].rearrange("p (h d) -> p h d", h=H)
                        nc.vector.tensor_tensor(
                            out=k3,
                            in0=k3,
                            in1=rk[:].unsqueeze(2).broadcast_to([128, H, DH]),
                            op=ALU.mult,
                        )
                        nc.sync.dma_start_transpose(
                            out=kT[:, :, t * 128:(t + 1) * 128], in_=ktok[:]
                        )

                    for s in range(2, NKT + 3):
                        if s < NKT:
                            ctx_stage_a(s)
                        if s < NKT + 2:
                            k_stage_b(s - 2)
                        if s >= 3:
                            k_stage_c(s - 3)

            # ---- P2b: attention per head, S/O software-pipelined ----
            wo_pool = stack.enter_context(tc.tile_pool(name="wo", bufs=1))
            wo_sb = wo_pool.tile([128, NF, DQ], BF16)
            for fi in range(NF):
                nc.sync.dma_start(
                    out=wo_sb[:, fi, :], in_=wo[fi * 128:(fi + 1) * 128, :]
                )
            with contextlib.ExitStack() as p2b:
                exp_pool = p2b.enter_context(tc.tile_pool(name="exp", bufs=4))
                rec_pool = p2b.enter_context(tc.tile_pool(name="rec", bufs=2))
                s_psum = p2b.enter_context(
                    tc.tile_pool(name="s_psum", bufs=2, space="PSUM")
                )
                ps_o_pool = p2b.enter_context(
                    tc.tile_pool(name="ps_o", bufs=2, space="PSUM")
                )
                ps_o_of = {}
                es_of = {}

                def emit_normalize(g):
                    """Denominator reciprocal + partition-broadcast + scale."""
                    gfi, gr = g // 2, (g % 2) * DH
                    ps_o = ps_o_of.pop(g)
                    rec_f = rec_pool.tile([1, LQS], F32, tag="rec_f")
                    nc.vector.reciprocal(out=rec_f[:], in_=ps_o[DH:DH + 1, :])
                    recr = rec_pool.tile([1, LQS], F32R, tag="recr")
                    nc.vector.tensor_copy(out=recr[:], in_=rec_f[:])
                    rb_ps = s_psum.tile([DH, LQS], F32, tag="mm_s")
                    for c in range(2):
                        nc.tensor.matmul(
                            rb_ps[:, c * 512:(c + 1) * 512],
                            ones_dh,
                            recr[0:1, c * 512:(c + 1) * 512],
                            start=True,
                            stop=True,
                        )
                    recb = rec_pool.tile([DH, LQS], F32, tag="recb")
                    nc.vector.tensor_copy(out=recb[:], in_=rb_ps[:])
                    nc.vector.tensor_tensor(
                        out=oaT[gr:gr + DH, gfi, :],
                        in0=ps_o[0:DH, :],
                        in1=recb[:],
                        op=ALU.mult,
                    )

                def emit_o(g, t):
                    gfi, gr = g // 2, (g % 2) * DH
                    for c in range(2):
                        nc.tensor.matmul(
                            ps_o_of[g][:, c * 512:(c + 1) * 512],
                            vpp[:, t, g, :],
                            es_of[(g, t)][:, c * 512:(c + 1) * 512],
                            start=(t == 0),
                            stop=(t == NKT - 1),
                        )
                    del es_of[(g, t)]

                for g in range(H):
                    gfi, gr = g // 2, (g % 2) * DH
                    ps_o_of[g] = ps_o_pool.tile(
                        [DH + 1, LQS], F32, tag="ps_o", name=f"ps_o{g}"
                    )
                    for t in range(NKT):
                        ps_s = s_psum.tile([128, LQS], F32, tag="mm_s")
                        for c in range(2):
                            nc.tensor.matmul(
                                ps_s[:, c * 512:(c + 1) * 512],
                                kT[gr:gr + DH, gfi, t * 128:(t + 1) * 128],
                                qhatT[gr:gr + DH, gfi, c * 512:(c + 1) * 512],
                                start=True,
                                stop=True,
                            )
                        es = exp_pool.tile([128, LQS], BF16, tag="es")
                        nc.scalar.activation(
                            out=es[:], in_=ps_s[:], func=AF.Exp, scale=inv_tau,
                        )
                        es_of[(g, t)] = es
                        if t >= 1:
                            emit_o(g, t - 1)
                        if t == 6 and g >= 1:
                            emit_normalize(g - 1)
                    emit_o(g, NKT - 1)
                emit_normalize(H - 1)

            # ---- P3: output projection ----
            with contextlib.ExitStack() as p3:
                mm_psum = p3.enter_context(
                    tc.tile_pool(name="mm_psum3", bufs=3, space="PSUM")
                )
                fin_pool = p3.enter_context(tc.tile_pool(name="fin", bufs=3))
                for t in range(NQT):
                    ft = fin_pool.tile([128, DQ], F32, tag="fin")
                    for n in range(2):
                        ps = mm_psum.tile([128, 512], F32, tag="mm")
                        for fi in range(NF):
                            nc.tensor.matmul(
                                ps[:],
                                oaT[:, fi, t * 128:(t + 1) * 128],
                                wo_sb[:, fi, n * 512:(n + 1) * 512],
                                start=(fi == 0),
                                stop=(fi == NF - 1),
                            )
                        nc.scalar.copy(
                            out=ft[:, n * 512:(n + 1) * 512], in_=ps[:]
                        )
                    nc.sync.dma_start(
                        out=out[t * 128:(t + 1) * 128, :], in_=ft[:]
                    )

    return nc


def split_multi_waits(nc):
    """walrus in this environment rejects >1 sync wait per instruction; move
    extras onto same-engine NOPs immediately preceding the instruction."""
    ctr = 0
    for f in nc.m.functions:
        for bb in f.blocks:
            new = []
            for inst in bb.instructions:
                si = inst.sync_info
                if si is not None and len(si.on_wait) > 1:
                    waits = list(si.on_wait)
                    for w in waits[:-1]:
                        nop = bass_rust.InstNoOp(name=f"I-wsplit-{ctr}")
                        ctr += 1
                        nop.engine = inst.engine
                        nop.sync_info = bass_rust.SyncInfo(
                            on_wait=[w], on_update=[]
                        )
                        new.append(nop)
                    inst.sync_info = bass_rust.SyncInfo(
                        on_wait=[waits[-1]], on_update=list(si.on_update)
                    )
                new.append(inst)
            bb.instructions[:] = new
    return ctr


_PROGRAM_CACHE = {}


def _get_program(inv_tau: float, has_bias: bool):
    key = (round(float(inv_tau), 12), has_bias)
    if key not in _PROGRAM_CACHE:
        nc = build_program(float(inv_tau), has_bias)
        split_multi_waits(nc)
        _PROGRAM_CACHE[key] = nc
    return _PROGRAM_CACHE[key]


def make_core_inputs(x, context, key_padding_mask, ln_q_w, ln_q_b, ln_ctx_w,
                     ln_ctx_b, Wq, Wk, Wv, Wo, tau):
    """Shard + host-side weight folding. Returns (in_maps, has_bias)."""
    import ml_dtypes

    f32 = np.float32
    bf16 = ml_dtypes.bfloat16
    x = np.asarray(x, f32)
    context = np.asarray(context, f32)
    mask01 = 1.0 - np.asarray(key_padding_mask).astype(f32)  # 1 = keep
    ln_q_w = np.asarray(ln_q_w, f32)
    ln_q_b = np.asarray(ln_q_b, f32)
    ln_ctx_w = np.asarray(ln_ctx_w, f32)
    ln_ctx_b = np.asarray(ln_ctx_b, f32)
    Wq = np.asarray(Wq, f32)
    Wk = np.asarray(Wk, f32)
    Wv = np.asarray(Wv, f32)
    Wo = np.asarray(Wo, f32)

    # fold LN affine into projections: (z*w + b) @ W = z @ (w*W) + b@W
    wq_f = np.ascontiguousarray(Wq * ln_q_w[:, None]).astype(bf16)
    wk_f = np.ascontiguousarray(Wk * ln_ctx_w[:, None]).astype(bf16)
    wv_f = np.ascontiguousarray(Wv * ln_ctx_w[:, None]).astype(bf16)
    wo_f = Wo.astype(bf16)
    has_bias = bool(np.any(ln_q_b != 0.0) or np.any(ln_ctx_b != 0.0))
    cq = (ln_q_b @ Wq).astype(f32)
    ck = (ln_ctx_b @ Wk).astype(f32)
    cv = (ln_ctx_b @ Wv).astype(f32)

    x_b = x.astype(bf16)
    ctx_b = context.astype(bf16)

    in_maps = []
    for core in range(8):
        b, hq = core // 2, core % 2
        m = {
            "xs": np.ascontiguousarray(x_b[b, hq * LQS:(hq + 1) * LQS, :]),
            "ctx": np.ascontiguousarray(ctx_b[b]),
            "mask01": np.ascontiguousarray(mask01[b]),
            "wq": wq_f,
            "wk": wk_f,
            "wv": wv_f,
            "wo": wo_f,
        }
        if has_bias:
            m["cq"], m["ck"], m["cv"] = cq, ck, cv
        in_maps.append(m)
    return in_maps, has_bias


def kernel(x, context, key_padding_mask, ln_q_w, ln_q_b, ln_ctx_w, ln_ctx_b,
           Wq, Wk, Wv, Wo, tau, _trace=False):
    in_maps, has_bias = make_core_inputs(
        x, context, key_padding_mask, ln_q_w, ln_q_b, ln_ctx_w, ln_ctx_b,
        Wq, Wk, Wv, Wo, tau,
    )
    inv_tau = 1.0 / (float(np.asarray(tau)) + 1e-6)
    nc = _get_program(inv_tau, has_bias)
    res = run_bass_kernel_spmd(nc, in_maps, list(range(8)), trace=_trace)
    out = np.empty((B, LQ, DQ), np.float32)
    for core in range(8):
        b, hq = core // 2, core % 2
        out[b, hq * LQS:(hq + 1) * LQS, :] = res.results[core]["out"]
    if _trace:
        return out, res
    return out
